# revision 1
# baseline (speedup 1.0000x reference)
"""AttentionSSA Trainium2 Bass kernel.

Computation (per batch b):
  qkv = x @ qkv_w + qkv_b ; split into per-head q,k,v
  S = (q @ k^T) * scale
  attn = softmax(w)[0] * softmax(S) + softmax(w)[1] * relu(S)^2
  out = (attn @ v) reassembled, @ proj_w + proj_b

Sharding: data-parallel over batch B=16 across 8 NeuronCores (2 batches/core).
Each core computes its slice fully independently (no collectives).

Per-core dataflow (matmuls in float32r, 1 cyc/row on the PE):
  P1: x_b [640,768] --PE transpose--> xT_b [768,640]
  P2: qT,kT = (qkv_w.T @ x.T) per feature tile (T-orientation, [feat, tok]);
      v in natural orientation [tok, feat] (lhsT = xT tiles), w0-prescaled.
      Biases are added via K=1 ones-row matmuls into the accumulating PSUM.
  P3: per (b, h): ST[ktok, qtok] = k q^T via lhsT/rhs slices of the qkT tiles;
      P0 = exp(SCALE*ST)  (ACT, fused scale)
      P1 = relu(sqrt(w1/w0)*SCALE*ST)^2  (ACT relu + TT square, bf16)
      Y0T(+denom row) = [w0*v | ones].T @ P0  (denom = col sums via M=65 trick)
      Y1T = v_bf16.T @ P1   (v_bf16 also w0-scaled; w1/w0 folded into P1)
      YT = Y0T * bcast(1/denom) + Y1T  (gpsimd partition_broadcast + 2 TT)
  P4: out = YT.T @ proj_w + proj_b  (lhsT = YT tiles directly), DMA out.
"""
import math
from contextlib import ExitStack

import numpy as np

import concourse.bacc as bacc
import concourse.bass as bass
import concourse.mybir as mybir
import concourse.tile as tile
from concourse.bass_utils import run_bass_kernel_spmd

F32 = mybir.dt.float32
F32R = mybir.dt.float32r
F16 = mybir.dt.float16
BF16 = mybir.dt.bfloat16
AF = mybir.ActivationFunctionType
ALU = mybir.AluOpType

NCORES = 8
B, N, D, H, DH = 16, 640, 768, 12, 64
BPC = B // NCORES          # batches per core
TOK = BPC * N              # tokens per core (1280)
SCALE = DH ** -0.5
KT = 5                     # 640/128 token tiles per batch
FT = 6                     # 768/128 dim tiles

# engine choice for PSUM->SBUF evictions ("scalar" = ACT, "vector" = DVE)
EV_XT = "vector"
EV_QK = "vector"
EV_PROJ = "vector"
SQ_GP_KT = (1, 3)          # kt indices whose relu2 square runs on gpsimd

# aux layout inside the combined f32r aux tile (column offsets)
A_ID = 0            # ident [128, 128]
A_ONESR = 128       # ones row [1, 640] (row 0)
A_ONESC = 768       # ones col [128, 1]
A_QKVB = 769        # qkv_b row [1, 2304]
A_PROJB = 3073      # proj_b row [1, 768]
A_ONES12 = 3841     # ones block [128, 12] (for v ones columns)
A_W = 3853          # total f32r aux cols
VW = H * (DH + 1)   # 780: per-(b,kt) v block: 12 heads x [64 feats | ones]


def _evict(nc, eng, dst, src):
    if eng == "scalar":
        nc.scalar.activation(dst, src, AF.Copy, bias=0.0, scale=1.0)
    else:
        nc.vector.tensor_copy(dst, src)


def build_nc(debug=False):
    nc = bacc.Bacc("TRN2", target_bir_lowering=False, debug=False)

    x_d = nc.dram_tensor("x", [TOK, D], F16, kind="ExternalInput")
    qkvw_d = nc.dram_tensor("qkv_w", [D, 3 * D], F16, kind="ExternalInput")
    projw_d = nc.dram_tensor("proj_w", [D, D], F16, kind="ExternalInput")
    auxr_d = nc.dram_tensor("auxr", [128, A_W], F16, kind="ExternalInput")
    auxf_d = nc.dram_tensor("auxf", [128, 2], F32, kind="ExternalInput")
    out_d = nc.dram_tensor("out", [TOK, D], F32, kind="ExternalOutput")
    if debug:
        dbg = {}
        for n, s in [("dbg_xt", [128, N]), ("dbg_q", [128, N]),
                     ("dbg_k", [128, N]), ("dbg_v", [128, VW]),
                     ("dbg_p0", [128, N]), ("dbg_p1", [128, N]),
                     ("dbg_yt", [128, N])]:
            dbg[n] = nc.dram_tensor(n, s, F16, kind="ExternalOutput")
        for n, s in [("dbg_st", [128, N]), ("dbg_av0", [65, N]),
                     ("dbg_av1", [64, N]), ("dbg_bc", [64, N])]:
            dbg[n] = nc.dram_tensor(n, s, F32, kind="ExternalOutput")

    with tile.TileContext(nc) as tc, ExitStack() as ctx:
        perm = ctx.enter_context(tc.tile_pool(name="perm", bufs=1))
        AX = perm.tile([128, A_W], F16, tag="auxr")
        AXF = perm.tile([128, 2], F32, tag="auxf")
        nc.sync.dma_start(AX[:], auxr_d[:])
        nc.sync.dma_start(AXF[:], auxf_d[:])
        ident = AX[:, A_ID:A_ID + 128]
        onesr = AX[0:1, A_ONESR:A_ONESR + N]
        onesc = AX[:, A_ONESC:A_ONESC + 1]
        qkvb = AX[0:1, A_QKVB:A_QKVB + 3 * D]
        projb = AX[0:1, A_PROJB:A_PROJB + D]
        w0c = AXF[:, 0:1]
        p1sc = AXF[:, 1:2]

        qv = ctx.enter_context(tc.tile_pool(name="pqv", bufs=1))
        QK = qv.tile([128, BPC * 12 * N], F16, tag="qk")  # (b,f): f<6 q, f>=6 k
        VR = qv.tile([128, BPC * KT * VW], F16, tag="vr")  # w0-scaled [v|1]

        def qk_col(b, f, c):
            return (b * 12 + f) * N + c

        def v_col(b, kt, c):
            return (b * KT + kt) * VW + c

        # ---------------- P1 + P2 per batch: xT_b, then qkT / v ----------------
        with tc.tile_pool(name="pwq", bufs=1) as pwq:
            WQ = pwq.tile([128, FT * 3 * D], F16, tag="wq")
            for k in range(FT):
                nc.sync.dma_start(WQ[:, k * 3 * D:(k + 1) * 3 * D],
                                  qkvw_d[k * 128:(k + 1) * 128, :])

            for b in range(BPC):
                with tc.tile_pool(name=f"pxt{b}", bufs=1) as pxt:
                    XT = pxt.tile([128, FT * N], F16, tag="xt")  # [dim, tok_b]

                    with tc.tile_pool(name=f"pxs{b}", bufs=1) as pxs, \
                         tc.tile_pool(name=f"ps1{b}", bufs=2, space="PSUM") as ps1:
                        XS = pxs.tile([128, KT * D], F16, tag="xs")
                        for t in range(KT):
                            nc.sync.dma_start(
                                XS[:, t * D:(t + 1) * D],
                                x_d[b * N + t * 128: b * N + (t + 1) * 128, :])
                        for ft in range(FT):
                            for g in range(0, KT, 4):
                                gw = min(4, KT - g) * 128
                                tp = ps1.tile([128, 512], F16, tag="tp")
                                for j in range(min(4, KT - g)):
                                    t = g + j
                                    nc.tensor.transpose(
                                        tp[:, j * 128:(j + 1) * 128],
                                        XS[:, t * D + ft * 128:
                                              t * D + (ft + 1) * 128],
                                        ident)
                                _evict(nc, EV_XT,
                                       XT[:, ft * N + g * 128:
                                             ft * N + g * 128 + gw],
                                       tp[:, 0:gw])

                    if debug and b == 0:
                        nc.sync.dma_start(dbg["dbg_xt"][:], XT[:, 0:N])
                    with tc.tile_pool(name=f"ps2a{b}", bufs=2, space="PSUM") as ps2a, \
                         tc.tile_pool(name=f"ps2b{b}", bufs=2, space="PSUM") as ps2b:
                        for f in range(12):
                            fcol = f * 128 if f < 6 else 768 + (f - 6) * 128
                            qp = ps2a.tile([128, N], F32, tag="qp")
                            for off, wd in ((0, 512), (512, 128)):
                                for k in range(FT):
                                    nc.tensor.matmul(
                                        qp[:, off:off + wd],
                                        WQ[:, k * 3 * D + fcol:
                                              k * 3 * D + fcol + 128],
                                        XT[:, k * N + off:
                                              k * N + off + wd],
                                        start=(k == 0), stop=False)
                                nc.tensor.matmul(
                                    qp[:, off:off + wd],
                                    qkvb[0:1, fcol:fcol + 128],
                                    onesr[0:1, 0:wd],
                                    start=False, stop=True)
                            _evict(nc, EV_QK,
                                   QK[:, qk_col(b, f, 0):qk_col(b, f, N)], qp[:])

                        for t in range(KT):
                            vp = ps2b.tile([128, D], F32, tag="vp")
                            for off, wd in ((0, 512), (512, 256)):
                                for k in range(FT):
                                    nc.tensor.matmul(
                                        vp[:, off:off + wd],
                                        XT[:, k * N + t * 128:
                                              k * N + (t + 1) * 128],
                                        WQ[:, k * 3 * D + 1536 + off:
                                              k * 3 * D + 1536 + off + wd],
                                        start=(k == 0), stop=False)
                                nc.tensor.matmul(
                                    vp[:, off:off + wd],
                                    onesr[0:1, 0:128],
                                    qkvb[0:1, 1536 + off:
                                              1536 + off + wd],
                                    start=False, stop=True)
                            vdst = VR[:, v_col(b, t, 0):v_col(b, t, VW)] \
                                .rearrange("p (h c) -> p h c", h=H)[:, :, 0:DH]
                            vsrc = vp[:].rearrange("p (h c) -> p h c", h=H)
                            nc.scalar.activation(vdst, vsrc,
                                                 AF.Copy, bias=0.0, scale=w0c)
                            vones = VR[:, v_col(b, t, 0):v_col(b, t, VW)] \
                                .rearrange("p (h c) -> p h c", h=H)[:, :, DH:DH + 1]
                            nc.scalar.activation(
                                vones,
                                AX[:, A_ONES12:A_ONES12 + H]
                                .rearrange("p (h c) -> p h c", c=1),
                                AF.Copy, bias=0.0, scale=1.0)

        if debug:
            nc.sync.dma_start(dbg["dbg_q"][:],
                              QK[:, qk_col(0, 0, 0):qk_col(0, 0, N)])
            nc.sync.dma_start(dbg["dbg_k"][:],
                              QK[:, qk_col(0, 6, 0):qk_col(0, 6, N)])
            nc.sync.dma_start(dbg["dbg_v"][:],
                              VR[:, v_col(0, 0, 0):v_col(0, 0, VW)])

        # ---------------- P3: attention per (b, h) ----------------
        with tc.tile_pool(name="pyt", bufs=1) as pyt:
            YT = pyt.tile([128, BPC * 6 * N], F16, tag="yt")

            def yt_col(b, pi, c):
                return (b * 6 + pi) * N + c

            with tc.tile_pool(name="pp0", bufs=6) as pp0, \
                 tc.tile_pool(name="ppr", bufs=3) as ppr, \
                 tc.tile_pool(name="pp1", bufs=6) as pp1, \
                 tc.tile_pool(name="psm", bufs=2) as psm, \
                 tc.tile_pool(name="ps3st", bufs=2, space="PSUM") as ps3st, \
                 tc.tile_pool(name="ps3a", bufs=1, space="PSUM") as ps3a, \
                 tc.tile_pool(name="ps3b", bufs=1, space="PSUM") as ps3b:
                for b in range(BPC):
                    for h in range(H):
                        pi, po = h // 2, 64 * (h % 2)
                        av0 = ps3a.tile([65, N], F32, tag="av0")
                        av1 = ps3b.tile([64, N], F32, tag="av1")
                        p0s, p1s_t = [], []
                        for kt in range(KT):
                            st = ps3st.tile([128, N], F32, tag="st")
                            for off, wd in ((0, 512), (512, 128)):
                                nc.tensor.matmul(
                                    st[:, off:off + wd],
                                    QK[po:po + 64,
                                       qk_col(b, 6 + pi, kt * 128):
                                       qk_col(b, 6 + pi, (kt + 1) * 128)],
                                    QK[po:po + 64,
                                       qk_col(b, pi, off):
                                       qk_col(b, pi, off + wd)],
                                    start=True, stop=True)
                            p0 = pp0.tile([128, N], F16, tag="p0")
                            nc.scalar.activation(p0[:], st[:], AF.Exp,
                                                 bias=0.0, scale=SCALE)
                            r = ppr.tile([128, N], F16, tag="r")
                            nc.scalar.activation(r[:], st[:], AF.Relu,
                                                 bias=0.0, scale=p1sc)
                            if debug and b == 0 and h == 0 and kt == 0:
                                _stb = psm.tile([128, N], F32, tag="dbgb")
                                nc.scalar.activation(_stb[:], st[:], AF.Copy,
                                                     bias=0.0, scale=1.0)
                                nc.sync.dma_start(dbg["dbg_st"][:], _stb[:])
                                nc.sync.dma_start(dbg["dbg_p0"][:], p0[:])
                            p1 = pp1.tile([128, N], F16, tag="p1")
                            if kt in SQ_GP_KT:
                                nc.gpsimd.tensor_tensor(p1[:], r[:], r[:],
                                                        ALU.mult)
                            else:
                                nc.vector.tensor_tensor(p1[:], r[:], r[:],
                                                        ALU.mult)
                            p0s.append(p0)
                            p1s_t.append(p1)
                        for kt in range(KT):
                            for off, wd in ((0, 512), (512, 128)):
                                sl = slice(off, off + wd)
                                nc.tensor.matmul(
                                    av0[0:65, sl],
                                    VR[:, v_col(b, kt, h * (DH + 1)):
                                          v_col(b, kt, h * (DH + 1) + DH + 1)],
                                    p0s[kt][:, sl],
                                    start=(kt == 0), stop=(kt == KT - 1))
                                nc.tensor.matmul(
                                    av1[0:64, sl],
                                    VR[:, v_col(b, kt, h * (DH + 1)):
                                          v_col(b, kt, h * (DH + 1) + DH)],
                                    p1s_t[kt][:, sl],
                                    start=(kt == 0), stop=(kt == KT - 1))
                        if debug and b == 0 and h == 0:
                            _a0 = psm.tile([65, N], F32, tag="dbga0")
                            nc.scalar.activation(_a0[:], av0[:], AF.Copy,
                                                 bias=0.0, scale=1.0)
                            nc.sync.dma_start(dbg["dbg_av0"][:], _a0[:])
                            _a1 = psm.tile([64, N], F32, tag="dbga1")
                            nc.scalar.activation(_a1[:], av1[0:64, :], AF.Copy,
                                                 bias=0.0, scale=1.0)
                            nc.sync.dma_start(dbg["dbg_av1"][:], _a1[:])
                        dln = psm.tile([1, N], F32, tag="dln")
                        nc.scalar.activation(dln[:], av0[64:65, :], AF.Ln,
                                             bias=0.0, scale=1.0)
                        drec = psm.tile([1, N], F32, tag="drec")
                        nc.scalar.activation(drec[:], dln[:], AF.Exp,
                                             bias=0.0, scale=-1.0)
                        bc = psm.tile([64, N], F32, tag="bc")
                        nc.gpsimd.partition_broadcast(bc[:], drec[:])
                        if debug and b == 0 and h == 0:
                            nc.sync.dma_start(dbg["dbg_bc"][:], bc[:])
                        tmp = psm.tile([64, N], F32, tag="tmp")
                        nc.vector.tensor_tensor(tmp[:], av0[0:64, :], bc[:],
                                                ALU.mult)
                        nc.vector.tensor_tensor(
                            YT[po:po + 64, yt_col(b, pi, 0):yt_col(b, pi, N)],
                            tmp[:], av1[0:64, :], ALU.add)

            if debug:
                nc.sync.dma_start(dbg["dbg_yt"][:],
                                  YT[:, yt_col(0, 0, 0):yt_col(0, 0, N)])
                nc.sync.dma_start(
                    dbg["dbg_p1"][:],
                    YT[:, yt_col(0, 1, 0):yt_col(0, 1, N)])

            # ---------------- P4: proj ----------------
            with tc.tile_pool(name="pw2", bufs=1) as pw2, \
                 tc.tile_pool(name="ps4", bufs=2, space="PSUM") as ps4:
                PW = pw2.tile([128, FT * D], F16, tag="pw")
                OUTS = pw2.tile([128, BPC * KT * D], F32, tag="outs")
                for k in range(FT):
                    nc.sync.dma_start(PW[:, k * D:(k + 1) * D],
                                      projw_d[k * 128:(k + 1) * 128, :])
                for b in range(BPC):
                    for t in range(KT):
                        op = ps4.tile([128, D], F32, tag="op")
                        for off, wd in ((0, 512), (512, 256)):
                            for f in range(FT):
                                nc.tensor.matmul(
                                    op[:, off:off + wd],
                                    YT[:, (b * 6 + f) * N + t * 128:
                                          (b * 6 + f) * N + (t + 1) * 128],
                                    PW[:, f * D + off:
                                          f * D + off + wd],
                                    start=(f == 0), stop=False)
                            nc.tensor.matmul(
                                op[:, off:off + wd],
                                onesr[0:1, 0:128],
                                projb[0:1, off:off + wd],
                                start=False, stop=True)
                        g = b * KT + t
                        _evict(nc, EV_PROJ, OUTS[:, g * D:(g + 1) * D], op[:])
                        nc.sync.dma_start(out_d[g * 128:(g + 1) * 128, :],
                                          OUTS[:, g * D:(g + 1) * D])

    nc.compile()
    return nc


_NC_CACHE = None


def _get_nc():
    global _NC_CACHE
    if _NC_CACHE is None:
        _NC_CACHE = build_nc()
    return _NC_CACHE


def kernel(x, qkv_w, qkv_b, proj_w, proj_b, w, t_h=8, t_w=8, s_h=24, s_w=24):
    x = np.asarray(x, dtype=np.float32)
    qkv_w = np.asarray(qkv_w, dtype=np.float32)
    qkv_b = np.asarray(qkv_b, dtype=np.float32)
    proj_w = np.asarray(proj_w, dtype=np.float32)
    proj_b = np.asarray(proj_b, dtype=np.float32)
    w = np.asarray(w, dtype=np.float32)

    we = np.exp(w - w.max())
    ws = we / we.sum()
    w0, w1 = float(ws[0]), float(ws[1])

    auxr = np.zeros((128, A_W), np.float32)
    auxr[:, A_ID:A_ID + 128] = np.eye(128, dtype=np.float32)
    auxr[0, A_ONESR:A_ONESR + N] = 1.0
    auxr[:, A_ONESC] = 1.0
    auxr[0, A_QKVB:A_QKVB + 3 * D] = qkv_b
    auxr[0, A_PROJB:A_PROJB + D] = proj_b
    auxr[:, A_ONES12:A_ONES12 + H] = 1.0
    auxf = np.zeros((128, 2), np.float32)
    auxf[:, 0] = w0
    auxf[:, 1] = math.sqrt(w1 / w0) * SCALE

    common = {"qkv_w": qkv_w.astype(np.float16),
              "proj_w": proj_w.astype(np.float16),
              "auxr": auxr.astype(np.float16), "auxf": auxf}
    in_maps = []
    for c in range(NCORES):
        m = dict(common)
        m["x"] = np.ascontiguousarray(
            x[c * BPC:(c + 1) * BPC].reshape(TOK, D)).astype(np.float16)
        in_maps.append(m)

    nc = _get_nc()
    res = run_bass_kernel_spmd(nc, in_maps, core_ids=list(range(NCORES)))
    out = np.concatenate(
        [r["out"].reshape(BPC, N, D) for r in res.results], axis=0)
    return out.astype(np.float32)

